# revision 21
# baseline (speedup 1.0000x reference)
"""Trainium2 Bass kernel for DecoderWithAttention (show-attend-tell decoder).

Strategy (8 NeuronCores, one chip):
 - Examples sorted by caption length (host), assigned round-robin to cores:
   global column c = 8j+k -> core k, local slot j. Each core owns 16 examples'
   encoder data (attention phase is owner-computes).
 - LSTM gate matmul G-sharded: core k computes gate rows {64k..64k+64} of each
   of i/f/g/o for ALL 128 examples; cell pointwise is G-sharded too; h-slices
   are AllGathered each step (h feature-major [512,128]).
 - ctx (attention readout) computed per-example on PE (alpha stationary M=1),
   gated, transposed on-chip, AllGathered feature-major for the gate matmul.
 - fc head V-sharded: core k computes logits columns [1250k, 1250(k+1)) for all
   examples from the gathered h; weights streamed per step (SBUF pressure).
 - All compute bf16 on PE with f32 PSUM; state h/c f32.
 - Inactive (example,step) outputs are zeroed on host (reference zeroes them).
"""

import os
import numpy as np
import ml_dtypes

import concourse.bass as bass
import concourse.mybir as mybir
import concourse.tile as tile
from concourse import bacc
from concourse.bass import ds
from concourse.bass_utils import run_bass_kernel_spmd
from concourse.masks import make_identity

B, P, ENC = 128, 196, 2048
L, V = 50, 10000
A, D, E = 512, 512, 512
T = L - 1          # 49 decode steps
NCORE = 8
EXC = B // NCORE   # 16 examples per core
VS = V // NCORE    # 1250 fc rows per core
VSP = 1280         # padded to 10*128
F = E + ENC + D    # 3072 x-features (emb | gated ctx | h)
GS = 4 * D // NCORE  # 256 gate rows per core (64 per gate)

bf16 = mybir.dt.bfloat16
f32 = mybir.dt.float32
nbf = ml_dtypes.bfloat16

LAST_EXEC_NS = None


def _build(nc, nown):
    AluOp = mybir.AluOpType
    Act = mybir.ActivationFunctionType
    ctxmgr = tile.TileContext(nc)

    # ---------------- I/O declarations ----------------
    d_in = {}
    def din(name, shape, dt):
        d_in[name] = nc.declare_dram_parameter(name, list(shape), dt, isOutput=False)
        return d_in[name]

    enc_lo_d = din("enc_lo", [128, EXC, ENC], bf16)       # p 0..127
    enc_hi_d = din("enc_hi", [68, EXC, ENC], bf16)        # p 128..195
    enc_se_d = din("enc_se", [16, 128, EXC * P], bf16)    # e-part setup layout
    enc_w_d = din("enc_w", [16, 128, A], bf16)            # enc_attn_W.T tiles
    enc_b_d = din("enc_b", [4, 128, 1], f32)
    mean_d = din("mean_t", [16, 128, B], bf16)            # enc_mean.T (all ex)
    init_w_d = din("init_w", [2, 16, 128, 64], bf16)      # h0/c0 weight slices
    init_b_d = din("init_b", [2, 64, 1], f32)
    dec_w_d = din("dec_w", [4, 128, A], bf16)             # dec_attn_W.T
    dec_b_d = din("dec_b", [4, 128, 1], f32)
    attw_d = din("attw", [128, 4], bf16)                  # full_attn_W chunks
    fbw_d = din("fbeta_w", [4, 128, ENC], bf16)           # fbeta_W.T
    fbb_d = din("fbeta_b", [128, 16, 1], f32)
    wall_d = din("wall", [24, 128, GS], bf16)             # [Wih|Whh].T G-slice
    gb_d = din("gates_b", [128, 2, 1], f32)               # bih+bhh slice
    fcw_d = din("fc_w", [10, 4, 128, 128], bf16)          # fc.T v-chunks padded
    fcb_d = din("fc_b", [128, 10, 1], f32)
    embt_d = din("embt", [T, 4, 128, B], bf16)            # emb.T per step, all ex
    msel_d = din("msel", [128, NCORE, 1], f32)            # one-hot rank select

    out_l = nc.declare_dram_parameter("out_l", [T, 10, 128, B], bf16, isOutput=True)
    alph_o = nc.declare_dram_parameter("alph", [EXC, T, P], f32, isOutput=True)

    RG = [list(range(NCORE))]

    with ctxmgr as tc:
        with (
            tc.tile_pool(name="res", bufs=1) as res,
            tc.tile_pool(name="dram", bufs=2, space="DRAM") as dram,
        ):
            # ---------------- persistent SBUF ----------------
            enc_lo = res.tile([128, EXC, ENC], bf16)
            enc_hi = res.tile([68, EXC, ENC], bf16)
            encf = res.tile([128, 4, EXC * P], bf16)      # enc_feat.T (a, (ex,p))
            dec_w = res.tile([128, 4, A], bf16)
            dec_b = res.tile([128, 4, 1], f32)
            attw = res.tile([128, 4], bf16)
            fbb = res.tile([128, 16, 1], f32)
            wall = res.tile([128, 24, GS], bf16)
            gb = res.tile([128, 2, 1], f32)
            fcb = res.tile([128, 10, 1], f32)
            ident = res.tile([16, 16], bf16)
            make_identity(nc, ident[:])
            hT_f = res.tile([128, 4, B], f32)
            hT_b = res.tile([128, 4, B], bf16)
            hT_own = res.tile([128, 4, EXC], bf16)
            c_st = res.tile([64, B], f32)                 # cell state slice
            hc_buf = res.tile([128, B], f32)              # rows 64:128 = h slice
            pw_if = res.tile([128, B], f32)               # i | f
            pw_go = res.tile([128, B], f32)               # g~ | o
            pw_f0 = res.tile([64, B], f32)                # f shifted to 0:64
            pw_t1 = res.tile([64, B], f32)
            tanh_hi = res.tile([128, B], f32)             # rows 64:128 tanh(c)
            dfT = res.tile([128, 4, EXC], f32)
            gateT = res.tile([128, 16, EXC], f32)
            ctxT = res.tile([128, 16, EXC], f32)
            ctxgT = res.tile([128, 16, EXC], bf16)
            ctxall = res.tile([128, 16, B], bf16)
            scores = res.tile([EXC, P], f32)
            nmax = res.tile([EXC, 1], f32)
            expv = res.tile([EXC, P], f32)
            sume = res.tile([EXC, 1], f32)
            rsum = res.tile([EXC, 1], f32)
            alpha_f = res.tile([EXC, P], f32)
            alpha_b = res.tile([EXC, P], bf16)
            alphaT = res.tile([128, 2, EXC], bf16)
            ctx_row = res.tile([32, 512], f32)
            ctx_tp = res.tile([32, 512], f32)

            msel = res.tile([128, NCORE, 1], f32)
            nc.sync.dma_start(msel[:], msel_d[:])

            # ---------------- setup phase ----------------
            nc.sync.dma_start(enc_lo[:], enc_lo_d[:])
            nc.sync.dma_start(enc_hi[:], enc_hi_d[:])
            nc.sync.dma_start(dec_w[:], dec_w_d.ap().rearrange("k p a -> p k a"))
            nc.sync.dma_start(dec_b[:], dec_b_d.ap().rearrange("k p o -> p k o"))
            nc.sync.dma_start(attw[:], attw_d[:])
            nc.sync.dma_start(fbb[:], fbb_d[:])
            nc.sync.dma_start(wall[:], wall_d.ap().rearrange("k p g -> p k g"))
            nc.sync.dma_start(gb[:], gb_d[:])
            nc.sync.dma_start(fcb[:], fcb_d[:])

            with (
                tc.tile_pool(name="setup", bufs=2) as sp,
                tc.tile_pool(name="spsum", bufs=1, space="PSUM") as psum,
            ):
                encb_sb = sp.tile([128, 4, 1], f32, tag="encb")
                nc.sync.dma_start(encb_sb[:], enc_b_d.ap().rearrange("k p o -> p k o"))
                # enc_feat.T = enc_attn_W' @ enc_s  (chunks of 448 over (ex,p))
                NCH = 7
                CW = EXC * P // NCH  # 448
                for ch in range(NCH):
                    pss4 = [psum.tile([128, CW], f32, tag=f"sps{at}",
                                      name=f"sps{at}")
                            for at in range(4)]
                    for kt in range(16):
                        encw_c = sp.tile([128, A], bf16, tag="encw")
                        nc.sync.dma_start(encw_c[:], enc_w_d[kt])
                        rhs = sp.tile([128, CW], bf16, tag="serhs")
                        nc.sync.dma_start(rhs[:],
                                          enc_se_d[kt, :, ch * CW:(ch + 1) * CW])
                        for at in range(4):
                            nc.tensor.matmul(
                                pss4[at][:], encw_c[:, at * 128:(at + 1) * 128],
                                rhs[:], start=(kt == 0), stop=(kt == 15))
                    for at in range(4):
                        nc.vector.tensor_scalar(
                            encf[:, at, ch * CW:(ch + 1) * CW], pss4[at][:],
                            encb_sb[:, at, :], None, AluOp.add)
                # h0 / c0 slices (mean and init weights streamed per k-tile)
                ib_sb = sp.tile([64, 2, 1], f32, tag="ib")
                nc.sync.dma_start(ib_sb[:], init_b_d.ap().rearrange("h d o -> d h o"))
                ps_h = psum.tile([64, B], f32, tag="h0ps")
                ps_c = psum.tile([64, B], f32, tag="c0ps")
                for kt in range(16):
                    mean_c = sp.tile([128, B], bf16, tag="meanc")
                    nc.sync.dma_start(mean_c[:], mean_d[kt])
                    iw_c = sp.tile([128, 2, 64], bf16, tag="iwc")
                    nc.sync.dma_start(iw_c[:],
                                      init_w_d[:, kt].rearrange("h p d -> p h d"))
                    nc.tensor.matmul(ps_h[:], iw_c[:, 0, :], mean_c[:],
                                     start=(kt == 0), stop=(kt == 15))
                    nc.tensor.matmul(ps_c[:], iw_c[:, 1, :], mean_c[:],
                                     start=(kt == 0), stop=(kt == 15))
                h0tmp = sp.tile([64, B], f32, tag="h0t")
                nc.scalar.activation(h0tmp[:], ps_h[:], Act.Identity,
                                     bias=ib_sb[:, 0, :])
                nc.sync.dma_start(hc_buf[64:128, :], h0tmp[:])
                nc.scalar.activation(c_st[:], ps_c[:], Act.Identity,
                                     bias=ib_sb[:, 1, :])

            # ---------------- decode loop ----------------
            from contextlib import ExitStack
            _ls = ExitStack()
            work = _ls.enter_context(tc.tile_pool(name="work", bufs=2))
            psum = _ls.enter_context(tc.tile_pool(name="lpsum", bufs=2, space="PSUM"))
            for t in range(T + 1):
                # --- AllGather h slices -> hT (feature-major [512, B]) ---
                h_in = dram.tile([64, B], f32, tag="h_in")
                h_out = dram.tile([NCORE * 64, B], f32, tag="h_out")
                nc.sync.dma_start(h_in[:], hc_buf[64:128, :])
                nc.gpsimd.collective_compute(
                    "AllGather", AluOp.bypass, replica_groups=RG,
                    ins=[h_in.opt()], outs=[h_out.opt()])
                nc.sync.dma_start(hT_f[:], h_out[:].rearrange("(k p) b -> p k b", p=128))
                for kt in range(4):
                    nc.vector.tensor_copy(hT_b[:, kt, :], hT_f[:, kt, :])
                nc.vector.tensor_scalar(hT_own[:], hT_b[:, :, 0:EXC],
                                        msel[:, 0, :], None, AluOp.mult)
                for r in range(1, NCORE):
                    nc.vector.scalar_tensor_tensor(
                        hT_own[:], hT_b[:, :, r * EXC:(r + 1) * EXC],
                        msel[:, r, :], hT_own[:], AluOp.mult, AluOp.add)

                # --- fc head for previous step (needs gathered h_t) ---
                if t > 0:
                    for vc in range(10):
                        fcw = work.tile([128, 4, 128], bf16, tag="fcw")
                        nc.sync.dma_start(fcw[:], fcw_d[vc].rearrange("k p v -> p k v"))
                        psf = psum.tile([128, B], f32, tag="gmm")
                        for kt in range(4):
                            nc.tensor.matmul(psf[:], fcw[:, kt, :], hT_b[:, kt, :],
                                             start=(kt == 0), stop=(kt == 3))
                        lg = work.tile([128, B], bf16, tag="lg")
                        nc.vector.tensor_scalar(lg[:], psf[:], fcb[:, vc, :], None,
                                                AluOp.add)
                        nc.sync.dma_start(out_l[t - 1, vc], lg[:])
                if t == T:
                    break

                # --- dec_feat.T [A, own16] ---
                for at in range(4):
                    psd = psum.tile([128, EXC], f32, tag="small")
                    for kt in range(4):
                        nc.tensor.matmul(psd[:], dec_w[:, kt, at * 128:(at + 1) * 128],
                                         hT_own[:, kt, :], start=(kt == 0), stop=(kt == 3))
                    nc.vector.tensor_scalar(dfT[:, at, :], psd[:], dec_b[:, at, :],
                                            None, AluOp.add)

                # --- gate.T = sigmoid(fbeta @ h_own)  [ENC, own16] ---
                for kt_e in range(16):
                    fbw_c = work.tile([128, 4, 128], bf16, tag="fbwc")
                    nc.sync.dma_start(
                        fbw_c[:],
                        fbw_d[:, :, kt_e * 128:(kt_e + 1) * 128]
                        .rearrange("k p e -> p k e"))
                    psg = psum.tile([128, EXC], f32, tag="small")
                    for kt in range(4):
                        nc.tensor.matmul(psg[:], fbw_c[:, kt, :],
                                         hT_own[:, kt, :], start=(kt == 0), stop=(kt == 3))
                    nc.scalar.activation(gateT[:, kt_e, :], psg[:], Act.Sigmoid,
                                         bias=fbb[:, kt_e, :])

                # --- relu(enc_feat + dec_feat) and scores, per own example ---
                # 4 examples per round via PE column-groups -> psum rows
                # {0,32,64,96}; DVE copies psum->SBUF (partition-preserving),
                # DMA then compacts rows into `scores`.
                na = nown[t]
                nrnd = (na + 3) // 4
                for rnd in range(nrnd):
                    pss = psum.tile([128, P], f32, tag="sps2")
                    for g in range(min(4, na - rnd * 4)):
                        ex = rnd * 4 + g
                        attn = work.tile([128, 4, P], bf16, tag="attn")
                        for at in range(4):
                            if at < 2:
                                nc.vector.tensor_scalar(
                                    attn[:, at, :], encf[:, at, ex * P:(ex + 1) * P],
                                    dfT[:, at, ex:ex + 1], 0.0, AluOp.add, AluOp.max)
                            else:
                                nc.scalar.activation(
                                    attn[:, at, :], encf[:, at, ex * P:(ex + 1) * P],
                                    Act.Relu, bias=dfT[:, at, ex:ex + 1])
                        for at in range(4):
                            nc.tensor.matmul(pss[32 * g:32 * g + 1, :],
                                             attw[:, at:at + 1], attn[:, at, :],
                                             start=(at == 0), stop=(at == 3),
                                             tile_position=(0, 32 * g))
                    stage_s = work.tile([128, P], f32, tag="stgs")
                    nc.vector.tensor_copy(stage_s[:], pss[:])
                    st4 = stage_s[:].rearrange("(g q) c -> g q c", q=32)[:, 0, :]
                    nc.sync.dma_start(scores[rnd * 4:(rnd + 1) * 4, :], st4)

                # --- softmax over P (rows = own examples) ---
                nc.vector.tensor_reduce(nmax[:], scores[:], mybir.AxisListType.X,
                                        AluOp.max, negate=True)
                nc.scalar.activation(expv[:], scores[:], Act.Exp, bias=nmax[:],
                                     accum_out=sume[:])
                nc.vector.reciprocal(rsum[:], sume[:])
                nc.vector.tensor_scalar(alpha_f[:], expv[:], rsum[:], None, AluOp.mult)
                nc.sync.dma_start(alph_o[0:na, t, :], alpha_f[0:na, :])
                nc.vector.tensor_copy(alpha_b[:], alpha_f[:])

                # --- alpha.T via PE transpose ---
                pst = psum.tile([128, EXC], bf16, tag="small")
                nc.tensor.transpose(pst[:], alpha_b[:, 0:128], ident[:])
                nc.vector.tensor_copy(alphaT[:, 0, :], pst[:])
                pst2 = psum.tile([68, EXC], bf16, tag="small")
                nc.tensor.transpose(pst2[:], alpha_b[:, 128:196], ident[:])
                nc.vector.tensor_copy(alphaT[0:68, 1, :], pst2[:])

                # --- ctx per example (alpha stationary, M=1), e in quarters ---
                for eh in range(4):
                    for rnd in range(nrnd):
                        psc = psum.tile([128, 512], f32, tag="cps")
                        for g in range(min(4, na - rnd * 4)):
                            ex = rnd * 4 + g
                            sl = slice(eh * 512, (eh + 1) * 512)
                            po = psc[32 * g:32 * g + 1, :]
                            nc.tensor.matmul(po, alphaT[:, 0, ex:ex + 1],
                                             enc_lo[:, ex, sl], start=True,
                                             stop=False, tile_position=(0, 32 * g))
                            nc.tensor.matmul(po, alphaT[0:68, 1, ex:ex + 1],
                                             enc_hi[:, ex, sl], start=False,
                                             stop=True, tile_position=(0, 32 * g))
                        stage_c = work.tile([128, 512], f32, tag="stgc")
                        if rnd % 2 == 0:
                            nc.vector.tensor_copy(stage_c[:], psc[:])
                        else:
                            nc.scalar.copy(stage_c[:], psc[:])
                        sc4 = stage_c[:].rearrange("(g q) c -> g q c", q=32)[:, 0, :]
                        nc.sync.dma_start(ctx_row[rnd * 4:(rnd + 1) * 4, :], sc4)
                    nc.vector.transpose(ctx_tp[:], ctx_row[:])
                    # scatter 32x32 blocks into ctxT tiles (4 DMAs per quarter)
                    src = ctx_tp[:].rearrange("r (m q c) -> r m q c", q=4, c=32)
                    for q in range(4):
                        nc.sync.dma_start(
                            ctxT[32 * q:32 * (q + 1), eh * 4:(eh + 1) * 4, :],
                            src[:, :, q, 0:EXC])
                # gate it
                nc.vector.tensor_tensor(ctxgT[:], ctxT[:], gateT[:], AluOp.mult)

                # --- AllGather gated ctx (feature-major) ---
                c_in = dram.tile([16, 128, EXC], bf16, tag="c_in")
                c_out = dram.tile([NCORE, 16, 128, EXC], bf16, tag="c_out")
                nc.sync.dma_start(c_in[:].rearrange("k p x -> p k x"), ctxgT[:])
                nc.gpsimd.collective_compute(
                    "AllGather", AluOp.bypass, replica_groups=RG,
                    ins=[c_in.opt()], outs=[c_out.opt()])
                for r in range(NCORE):
                    nc.sync.dma_start(
                        ctxall[:, :, r * EXC:(r + 1) * EXC],
                        c_out[r].rearrange("k p x -> p k x"))

                # --- G-sharded gate matmul over x = [emb | ctxg | h] ---
                emb_sb = work.tile([128, 4, B], bf16, tag="emb")
                nc.sync.dma_start(emb_sb[:], embt_d[t].rearrange("k p b -> p k b"))
                psG = []
                for mt in range(2):
                    pg = psum.tile([128, B], f32, tag="gmm")
                    for kt in range(24):
                        if kt < 4:
                            rhs = emb_sb[:, kt, :]
                        elif kt < 20:
                            rhs = ctxall[:, kt - 4, :]
                        else:
                            rhs = hT_b[:, kt - 20, :]
                        nc.tensor.matmul(pg[:], wall[:, kt, mt * 128:(mt + 1) * 128],
                                         rhs, start=(kt == 0), stop=(kt == 23))
                    psG.append(pg)
                # pointwise cell update: Mt0 = [i | f], Mt1 = [g~ | o]
                nc.scalar.activation(pw_if[0:64, :], psG[0][0:64, :], Act.Sigmoid,
                                     bias=gb[0:64, 0, :])
                nc.scalar.activation(pw_if[64:128, :], psG[0][64:128, :], Act.Sigmoid,
                                     bias=gb[64:128, 0, :])
                nc.scalar.activation(pw_go[0:64, :], psG[1][0:64, :], Act.Tanh,
                                     bias=gb[0:64, 1, :])
                nc.scalar.activation(pw_go[64:128, :], psG[1][64:128, :], Act.Sigmoid,
                                     bias=gb[64:128, 1, :])
                nc.sync.dma_start(pw_f0[:], pw_if[64:128, :])       # shift f down
                nc.vector.tensor_tensor(pw_t1[:], pw_if[0:64, :], pw_go[0:64, :],
                                        AluOp.mult)                 # i*g~
                nc.vector.tensor_tensor(c_st[:], pw_f0[:], c_st[:], AluOp.mult)
                nc.vector.tensor_tensor(c_st[:], c_st[:], pw_t1[:], AluOp.add)
                nc.scalar.activation(pw_t1[:], c_st[:], Act.Tanh)
                nc.sync.dma_start(tanh_hi[64:128, :], pw_t1[:])     # shift tanh up
                nc.vector.tensor_tensor(hc_buf[64:128, :], pw_go[64:128, :],
                                        tanh_hi[64:128, :], AluOp.mult)
            _ls.close()
    return nc


def _prep_inputs(inputs):
    enc = np.asarray(inputs["encoder_out"], np.float32)
    cap = np.asarray(inputs["enc_cap"], np.int32)
    cl = np.asarray(inputs["cap_len"], np.int32)
    sort_ind = np.argsort(-cl, kind="stable").astype(np.int32)
    cap_s = cl[sort_ind]
    dec_len = (cap_s - 1).astype(np.int32)
    caps_s = cap[sort_ind]
    enc_s = enc[sort_ind]                       # [128, 196, 2048]
    emb = np.asarray(inputs["emb_W"], np.float32)[caps_s]   # [128, 50, 512]

    W = {k: np.asarray(v, np.float32) for k, v in inputs.items()
         if k not in ("encoder_out", "enc_cap", "cap_len")}

    cp = np.arange(B)
    gperm = NCORE * (cp % EXC) + cp // EXC      # device col c' -> sorted example
    mean_t = enc_s.mean(axis=1).T[:, gperm].astype(nbf)   # [2048, 128]
    mean_t = mean_t.reshape(16, 128, B)

    wih, whh = W["lstm_Wih"], W["lstm_Whh"]     # [2048, 2560], [2048, 512]
    wall_full = np.concatenate([wih, whh], axis=1)  # [2048, 3072]
    gbias = W["lstm_bih"] + W["lstm_bhh"]

    et = emb[:, :T, :].transpose(1, 2, 0)[:, :, gperm]    # [T, 512, 128]
    embt = et.reshape(T, 4, 128, B).astype(nbf)

    per_core = []
    for k in range(NCORE):
        own = np.arange(EXC) * NCORE + k        # global columns 8j+k
        a = enc_s[own]                          # [16, 196, 2048]
        enc_lo = np.ascontiguousarray(a[:, :128, :].transpose(1, 0, 2)).astype(nbf)
        enc_hi = np.ascontiguousarray(a[:, 128:, :].transpose(1, 0, 2)).astype(nbf)
        # e-part layout: [kt, e%128, ex*196+p]
        se = a.transpose(2, 0, 1).reshape(16, 128, EXC * P).astype(nbf)

        grows = np.concatenate([g * D + np.arange(64 * k, 64 * (k + 1))
                                for g in range(4)])
        wsl = wall_full[grows].T                # [3072, 256]
        wall_t = wsl.reshape(24, 128, GS).astype(nbf)
        gb_sl = gbias[grows].astype(np.float32) # [256]
        gb_a = gb_sl.reshape(2, 128, 1).transpose(1, 0, 2)  # [128,2,1]

        vrows = np.arange(VS * k, VS * (k + 1))
        fcw = np.zeros((VSP, D), np.float32)
        fcw[:VS] = W["fc_W"][vrows]
        fcb = np.zeros((VSP,), np.float32)
        fcb[:VS] = W["fc_b"][vrows]
        fcw_t = fcw.T.reshape(4, 128, 10, 128).transpose(2, 0, 1, 3).astype(nbf)
        fcb_a = fcb.reshape(10, 128, 1).transpose(1, 0, 2)

        iw = np.stack([W["init_h_W"][64 * k:64 * (k + 1)].T,
                       W["init_c_W"][64 * k:64 * (k + 1)].T])  # [2, 2048, 64]
        iw_t = iw.reshape(2, 16, 128, 64).astype(nbf)
        ib = np.stack([W["init_h_b"][64 * k:64 * (k + 1)],
                       W["init_c_b"][64 * k:64 * (k + 1)]]).reshape(2, 64, 1)

        m = {
            "enc_lo": enc_lo, "enc_hi": enc_hi, "enc_se": se,
            "enc_w": W["enc_attn_W"].T.reshape(16, 128, A).astype(nbf),
            "enc_b": W["enc_attn_b"].reshape(4, 128, 1).astype(np.float32),
            "mean_t": mean_t,
            "init_w": iw_t, "init_b": ib.astype(np.float32),
            "dec_w": W["dec_attn_W"].T.reshape(4, 128, A).astype(nbf),
            "dec_b": W["dec_attn_b"].reshape(4, 128, 1).astype(np.float32),
            "attw": W["full_attn_W"][0].reshape(4, 128).T.astype(nbf),
            "fbeta_w": W["fbeta_W"].T.reshape(4, 128, ENC).astype(nbf),
            "fbeta_b": W["fbeta_b"].reshape(16, 128, 1).transpose(1, 0, 2)
                        .astype(np.float32),
            "wall": wall_t, "gates_b": gb_a.astype(np.float32),
            "fc_w": fcw_t, "fc_b": fcb_a.astype(np.float32),
            "embt": embt,
            "msel": np.tile(np.eye(NCORE, dtype=np.float32)[k].reshape(1, NCORE, 1),
                            (128, 1, 1)),
        }
        m = {k2: np.ascontiguousarray(v) for k2, v in m.items()}
        per_core.append(m)
    return per_core, sort_ind, caps_s, dec_len


def kernel(**inputs):
    global LAST_EXEC_NS
    per_core, sort_ind, caps_s, dec_len = _prep_inputs(inputs)

    bs = [int((dec_len > t).sum()) for t in range(T)]
    nown = [max(1, (b + NCORE - 1) // NCORE) for b in bs]

    nc = bacc.Bacc(None, target_bir_lowering=False)
    _build(nc, nown)
    if not nc.is_finalized():
        nc.finalize()

    trace = os.environ.get("KERNEL_TRACE", "0") == "1"
    import time as _time
    _t0 = _time.time()
    res = run_bass_kernel_spmd(nc, per_core, core_ids=list(range(NCORE)),
                               trace=trace)
    globals()["LAST_WALL_S"] = _time.time() - _t0
    LAST_EXEC_NS = res.exec_time_ns
    if os.environ.get("KERNEL_BENCH", "0") == "1":
        import time as _time
        ts = []
        for _ in range(3):
            t0 = _time.time()
            run_bass_kernel_spmd(nc, per_core, core_ids=list(range(NCORE)))
            ts.append(_time.time() - t0)
        print(f"bench wall times: {[f'{x*1e3:.1f}ms' for x in ts]}", flush=True)

    out = np.zeros((B, T, V), np.float32)
    alphas = np.zeros((B, T, P), np.float32)
    tgrid = np.arange(T)[None, :]
    act = (tgrid < dec_len[:, None]).astype(np.float32)   # [B, T]
    for k in range(NCORE):
        r = res.results[k]
        lg = np.asarray(r["out_l"]).astype(np.float32)     # [T,10,128,B]
        lg = lg.transpose(3, 0, 1, 2).reshape(B, T, VSP)[:, :, :VS]
        cp = np.arange(B)
        gperm = NCORE * (cp % EXC) + cp // EXC
        out[gperm, :, VS * k:VS * (k + 1)] = lg
        al = np.asarray(r["alph"]).astype(np.float32)      # [16, T, P]
        own = np.arange(EXC) * NCORE + k
        alphas[own] = al
    out *= act[:, :, None]
    alphas *= act[:, :, None]
    return (out, caps_s.astype(np.int32), dec_len.astype(np.int32),
            alphas, sort_ind.astype(np.int32))
